# revision 35
# baseline (speedup 1.0000x reference)
"""DualPathFusion TRN2 kernel.

Reference computes, per spatial position x (with C=32 channels):
    avg = (f1 + f2) / 2
    a1  = w1[:C] . f1[:, x] + w1[C:] . avg[:, x] + b1
    a2  = w2[:C] . f2[:, x] + w2[C:] . avg[:, x] + b2
    s   = softmax([a1, a2])           # over the 2 logits
    out = f1 * s[0] + f2 * s[1]

Softmax over two logits is sigmoid of the difference, and avg is a linear
mix of f1/f2, so with
    u = w1[:C] + (w1[C:] - w2[C:]) / 2
    v = -w2[:C] + (w1[C:] - w2[C:]) / 2
    d = u . f1[:, x] + v . f2[:, x] + (b1 - b2)
the output is exactly  out = f2 + (f1 - f2) * sigmoid(d).

Distribution: pure data parallel over 8 cores — batch (2) x spatial
quarter (4). Each core handles a [32, 131072] slab of f1/f2.

Host-side prep re-packs each core's slab into tile-major layout
[n_tiles, 128, tile_n] (partition p = chunk j * 32 + channel c), so every
device DMA is one fully contiguous 1 MiB stream — strided 8 KB descriptor
patterns only reach ~45 GB/s of HBM bandwidth, contiguous ~340 GB/s.

On-chip: block-diagonal [128, 128] weight matrices (u / v replicated
across the 32 columns of each diagonal block) make
  PSUM[p, n] = sum_k U[k, p] * f1_tile[k, n] + sum_k V[k, p] * f2_tile[k, n]
equal d for chunk p//32 on every partition p — the channel contraction
lands in PSUM already broadcast across channels.  The scalar engine
applies sigmoid(+bias) PSUM->SBUF; GPSIMD computes f1-f2; the vector
engine does the final two elementwise ops.  Matmuls run as float32r
(same 32-bit data, 4x faster PE streaming than fp32).
"""

import numpy as np

import concourse.bacc as bacc
import concourse.bass as bass
import concourse.mybir as mybir
import concourse.tile as tile
from concourse.bass_utils import run_bass_kernel_spmd

B, C, D, H, W = 2, 32, 32, 128, 128
S = D * H * W                  # 524288 spatial positions per batch
N_CORES = 8
QUARTERS = N_CORES // B        # spatial quarters per batch -> 8 shards
S_CORE = S // QUARTERS         # 131072 positions per core
P = 128                        # SBUF partitions
NCHUNK = P // C                # 4 spatial chunks packed into partitions
TILE_N = 2048                  # positions per on-chip tile column block
N_TILES = S_CORE // NCHUNK // TILE_N   # 16
FP32 = mybir.dt.float32

# Exposed for test harnesses: the BassKernelResults of the last run.
LAST_RESULTS = None


def build_nc(n_tiles: int = N_TILES, tile_n: int = TILE_N, mm_n: int = 512,
             io_bufs: int = 4, work_bufs: int = 3,
             psum_bufs: int = 6, mm_fp32r: bool = True):
    """Build the per-core Bass program (same program on all 8 cores)."""
    assert tile_n % mm_n == 0

    # Bacc (not plain Bass): its compile pipeline splits multi-sem waits
    # and moves matmul waits to ldweights — walrus codegen rejects
    # instructions with >1 sync wait otherwise.
    nc = bacc.Bacc("TRN2", target_bir_lowering=False)
    FP32R = mybir.dt.float32r
    # float32r: identical 32-bit storage, but the PE streams fp32r matmuls
    # at 1 col/cycle instead of fp32's 2x half-speed decomposition. The BIR
    # verifier requires fp32r matmul operands to be *produced* as fp32r, so
    # the feature tensors are fp32r end-to-end; elementwise engines read
    # them through a bitcast back to plain fp32.
    mm_dt = FP32R if mm_fp32r else FP32
    f1 = nc.dram_tensor("f1", [n_tiles, P, tile_n], mm_dt, kind="ExternalInput")
    f2 = nc.dram_tensor("f2", [n_tiles, P, tile_n], mm_dt, kind="ExternalInput")
    # one packed const tensor: [u128 | v128 | bias] -> a single small DMA
    cc = nc.dram_tensor("cc", [P, 2 * P + 1], mm_dt, kind="ExternalInput")
    out = nc.dram_tensor("out", [n_tiles, P, tile_n], FP32,
                         kind="ExternalOutput")

    sig = mybir.ActivationFunctionType.Sigmoid

    def ew_ap(ap):
        # plain-fp32 view for the elementwise engines
        return ap.bitcast(FP32) if mm_fp32r else ap

    with tile.TileContext(nc) as tc:
        with (
            tc.tile_pool(name="const", bufs=1) as cpool,
            tc.tile_pool(name="io", bufs=io_bufs) as io,
            tc.tile_pool(name="work", bufs=work_bufs) as work,
            tc.tile_pool(name="psum", bufs=psum_bufs, space="PSUM") as pp,
        ):
            c_t = cpool.tile([P, 2 * P + 1], mm_dt, tag="c")
            # single tiny const DMA ahead of the t1 stream (four separate
            # const DMAs took ~15 us to land and gated the first matmul)
            nc.sync.dma_start(c_t[:], cc[:])
            u_t = c_t[:, 0:P]
            v_t = c_t[:, P:2 * P]
            b_t = c_t[:, 2 * P:2 * P + 1].bitcast(FP32)

            # Three DMA dispatch paths (SP HWDGE ring, Activation HWDGE
            # ring, GPSIMD SWDGE). The SDMA engines round-robin across
            # ACTIVE queues, so one queue sustains only ~145 GB/s. Keep
            # roles DEDICATED (mixing loads and stores on one ring makes
            # load dispatches queue behind store-data waits — head-of-line
            # blocking), but borrow idle paths at the edges: the SWDGE path
            # helps with two loads during ramp-up, and the two HWDGE rings
            # absorb the last two stores during the drain.
            def emit_chunk(i, c0, w, r_t1, r_t2, r_st):
                """Load/compute/store one [P, w] slice of DRAM tile i."""
                t1 = io.tile([P, w], mm_dt, tag="t1")
                t2 = io.tile([P, w], mm_dt, tag="t2")
                r_t1.dma_start(t1[:], f1[i][:, c0:c0 + w])
                r_t2.dma_start(t2[:], f2[i][:, c0:c0 + w])

                s_t = work.tile([P, w], FP32, tag="s")
                for k in range(w // mm_n):
                    ks = bass.ts(k, mm_n)
                    d_ps = pp.tile([P, mm_n], FP32, tag="d")
                    nc.tensor.matmul(d_ps[:], u_t, t1[:, ks],
                                     start=True, stop=False)
                    nc.tensor.matmul(d_ps[:], v_t, t2[:, ks],
                                     start=False, stop=True)
                    # s = sigmoid(d + (b1 - b2)), PSUM -> SBUF
                    nc.scalar.activation(s_t[:, ks], d_ps[:], sig,
                                         bias=b_t)

                df = work.tile([P, w], FP32, tag="df")
                o_t = work.tile([P, w], FP32, tag="o")
                # sub on the (otherwise idle) GPSIMD engine; DVE does the
                # remaining two tensor_tensor ops.
                nc.gpsimd.tensor_sub(df[:], ew_ap(t1[:]), ew_ap(t2[:]))
                nc.vector.tensor_mul(o_t[:], df[:], s_t[:])
                nc.vector.tensor_add(o_t[:], o_t[:], ew_ap(t2[:]))

                r_st.dma_start(out[i][:, c0:c0 + w], o_t[:])

            for i in range(n_tiles):
                # loads dedicated to the two HWDGE rings; the idle SWDGE
                # path absorbs two mid-ramp t2 loads (stores haven't
                # started yet) and the two HWDGE rings absorb the last
                # two stores during the drain (loads are done by then)
                r_t1 = nc.sync
                r_t2 = nc.scalar
                if i == n_tiles - 2:
                    r_st = nc.sync
                elif i == n_tiles - 1:
                    r_st = nc.scalar
                else:
                    r_st = nc.gpsimd
                emit_chunk(i, 0, tile_n, r_t1, r_t2, r_st)
    nc.finalize()
    return nc


def make_weights(w1, b1, w2, b2):
    """Host-side prep of the tiny conv weights (128 floats -> 2x[128,128])."""
    c = C
    wd = 0.5 * (w1[c:] - w2[c:])
    u = (w1[:c] + wd).astype(np.float32)
    v = (-w2[:c] + wd).astype(np.float32)
    u128 = np.zeros((P, P), np.float32)
    v128 = np.zeros((P, P), np.float32)
    for j in range(NCHUNK):
        blk = slice(j * c, (j + 1) * c)
        u128[blk, blk] = u[:, None]    # rows k = contraction, cols = bcast
        v128[blk, blk] = v[:, None]
    bias = np.full((P, 1), np.float32(b1[0]) - np.float32(b2[0]), np.float32)
    return u128, v128, bias


def to_tile_major(slab):
    """[C, S_CORE] -> [N_TILES, 128, TILE_N] with partition p = j*32 + c."""
    # (c, j, i, n) -> (i, jc, n)
    x = slab.reshape(C, NCHUNK, N_TILES, TILE_N)
    return np.ascontiguousarray(x.transpose(2, 1, 0, 3)).reshape(
        N_TILES, P, TILE_N)


def from_tile_major(tiles):
    """Inverse of to_tile_major."""
    x = tiles.reshape(N_TILES, NCHUNK, C, TILE_N)
    return np.ascontiguousarray(x.transpose(2, 1, 0, 3)).reshape(C, S_CORE)


def kernel(feature1, feature2, w1, b1, w2, b2):
    global LAST_RESULTS
    u128, v128, bias = make_weights(w1, b1, w2, b2)

    f1v = np.asarray(feature1, np.float32).reshape(B, C, S)
    f2v = np.asarray(feature2, np.float32).reshape(B, C, S)

    in_maps = []
    for k in range(N_CORES):
        b, q = divmod(k, QUARTERS)
        sl = slice(q * S_CORE, (q + 1) * S_CORE)
        in_maps.append({
            "f1": to_tile_major(f1v[b, :, sl]),
            "f2": to_tile_major(f2v[b, :, sl]),
            "cc": np.concatenate([u128, v128, bias], axis=1),
        })

    nc = build_nc()
    res = run_bass_kernel_spmd(nc, in_maps, list(range(N_CORES)))
    LAST_RESULTS = res

    shards = np.stack([from_tile_major(res.results[k]["out"])
                       for k in range(N_CORES)])
    full = (shards.reshape(B, QUARTERS, C, S_CORE)
                  .transpose(0, 2, 1, 3)
                  .reshape(B, C, D, H, W))
    return full.astype(np.float32)


# revision 36
# speedup vs baseline: 1.0495x; 1.0495x over previous
"""DualPathFusion TRN2 kernel.

Reference computes, per spatial position x (with C=32 channels):
    avg = (f1 + f2) / 2
    a1  = w1[:C] . f1[:, x] + w1[C:] . avg[:, x] + b1
    a2  = w2[:C] . f2[:, x] + w2[C:] . avg[:, x] + b2
    s   = softmax([a1, a2])           # over the 2 logits
    out = f1 * s[0] + f2 * s[1]

Softmax over two logits is sigmoid of the difference, and avg is a linear
mix of f1/f2, so with
    u = w1[:C] + (w1[C:] - w2[C:]) / 2
    v = -w2[:C] + (w1[C:] - w2[C:]) / 2
    d = u . f1[:, x] + v . f2[:, x] + (b1 - b2)
the output is exactly  out = f2 + (f1 - f2) * sigmoid(d).

Distribution: pure data parallel over 8 cores — batch (2) x spatial
quarter (4). Each core handles a [32, 131072] slab of f1/f2.

Host-side prep re-packs each core's slab into tile-major layout
[n_tiles, 128, tile_n] (partition p = chunk j * 32 + channel c), so every
device DMA is one fully contiguous 1 MiB stream — strided 8 KB descriptor
patterns only reach ~45 GB/s of HBM bandwidth, contiguous ~340 GB/s.

On-chip: block-diagonal [128, 128] weight matrices (u / v replicated
across the 32 columns of each diagonal block) make
  PSUM[p, n] = sum_k U[k, p] * f1_tile[k, n] + sum_k V[k, p] * f2_tile[k, n]
equal d for chunk p//32 on every partition p — the channel contraction
lands in PSUM already broadcast across channels.  The scalar engine
applies sigmoid(+bias) PSUM->SBUF; GPSIMD computes f1-f2; the vector
engine does the final two elementwise ops.  Matmuls run as float32r
(same 32-bit data, 4x faster PE streaming than fp32).
"""

import numpy as np

import concourse.bacc as bacc
import concourse.bass as bass
import concourse.mybir as mybir
import concourse.tile as tile
from concourse.bass_utils import run_bass_kernel_spmd

B, C, D, H, W = 2, 32, 32, 128, 128
S = D * H * W                  # 524288 spatial positions per batch
N_CORES = 8
QUARTERS = N_CORES // B        # spatial quarters per batch -> 8 shards
S_CORE = S // QUARTERS         # 131072 positions per core
P = 128                        # SBUF partitions
NCHUNK = P // C                # 4 spatial chunks packed into partitions
TILE_N = 2048                  # positions per on-chip tile column block
N_TILES = S_CORE // NCHUNK // TILE_N   # 16
FP32 = mybir.dt.float32

# Exposed for test harnesses: the BassKernelResults of the last run.
LAST_RESULTS = None


def build_nc(n_tiles: int = N_TILES, tile_n: int = TILE_N, mm_n: int = 512,
             io_bufs: int = 5, work_bufs: int = 4,
             psum_bufs: int = 8, mm_fp32r: bool = True):
    """Build the per-core Bass program (same program on all 8 cores)."""
    assert tile_n % mm_n == 0

    # Bacc (not plain Bass): its compile pipeline splits multi-sem waits
    # and moves matmul waits to ldweights — walrus codegen rejects
    # instructions with >1 sync wait otherwise.
    nc = bacc.Bacc("TRN2", target_bir_lowering=False)
    FP32R = mybir.dt.float32r
    # float32r: identical 32-bit storage, but the PE streams fp32r matmuls
    # at 1 col/cycle instead of fp32's 2x half-speed decomposition. The BIR
    # verifier requires fp32r matmul operands to be *produced* as fp32r, so
    # the feature tensors are fp32r end-to-end; elementwise engines read
    # them through a bitcast back to plain fp32.
    mm_dt = FP32R if mm_fp32r else FP32
    f1 = nc.dram_tensor("f1", [n_tiles, P, tile_n], mm_dt, kind="ExternalInput")
    f2 = nc.dram_tensor("f2", [n_tiles, P, tile_n], mm_dt, kind="ExternalInput")
    # one packed const tensor: [u128 | v128 | bias] -> a single small DMA
    cc = nc.dram_tensor("cc", [P, 2 * P + 1], mm_dt, kind="ExternalInput")
    out = nc.dram_tensor("out", [n_tiles, P, tile_n], FP32,
                         kind="ExternalOutput")

    sig = mybir.ActivationFunctionType.Sigmoid

    def ew_ap(ap):
        # plain-fp32 view for the elementwise engines
        return ap.bitcast(FP32) if mm_fp32r else ap

    with tile.TileContext(nc) as tc:
        with (
            tc.tile_pool(name="const", bufs=1) as cpool,
            tc.tile_pool(name="io", bufs=io_bufs) as io,
            tc.tile_pool(name="work", bufs=work_bufs) as work,
            tc.tile_pool(name="psum", bufs=psum_bufs, space="PSUM") as pp,
        ):
            c_t = cpool.tile([P, 2 * P + 1], mm_dt, tag="c")
            # single tiny const DMA ahead of the t1 stream (four separate
            # const DMAs took ~15 us to land and gated the first matmul)
            nc.sync.dma_start(c_t[:], cc[:])
            u_t = c_t[:, 0:P]
            v_t = c_t[:, P:2 * P]
            b_t = c_t[:, 2 * P:2 * P + 1].bitcast(FP32)

            # Three DMA dispatch paths (SP HWDGE ring, Activation HWDGE
            # ring, GPSIMD SWDGE). The SDMA engines round-robin across
            # ACTIVE queues, so one queue sustains only ~145 GB/s. Keep
            # roles DEDICATED (mixing loads and stores on one ring makes
            # load dispatches queue behind store-data waits — head-of-line
            # blocking), but borrow idle paths at the edges: the SWDGE path
            # helps with two loads during ramp-up, and the two HWDGE rings
            # absorb the last two stores during the drain.
            def emit_chunk(i, c0, w, r_t1, r_t2, r_st):
                """Load/compute/store one [P, w] slice of DRAM tile i."""
                t1 = io.tile([P, w], mm_dt, tag="t1")
                t2 = io.tile([P, w], mm_dt, tag="t2")
                r_t1.dma_start(t1[:], f1[i][:, c0:c0 + w])
                r_t2.dma_start(t2[:], f2[i][:, c0:c0 + w])

                s_t = work.tile([P, w], FP32, tag="s")
                for k in range(w // mm_n):
                    ks = bass.ts(k, mm_n)
                    d_ps = pp.tile([P, mm_n], FP32, tag="d")
                    nc.tensor.matmul(d_ps[:], u_t, t1[:, ks],
                                     start=True, stop=False)
                    nc.tensor.matmul(d_ps[:], v_t, t2[:, ks],
                                     start=False, stop=True)
                    # s = sigmoid(d + (b1 - b2)), PSUM -> SBUF
                    nc.scalar.activation(s_t[:, ks], d_ps[:], sig,
                                         bias=b_t)

                df = work.tile([P, w], FP32, tag="df")
                o_t = work.tile([P, w], FP32, tag="o")
                # sub on the (otherwise idle) GPSIMD engine; DVE does the
                # remaining two tensor_tensor ops.
                nc.gpsimd.tensor_sub(df[:], ew_ap(t1[:]), ew_ap(t2[:]))
                nc.vector.tensor_mul(o_t[:], df[:], s_t[:])
                nc.vector.tensor_add(o_t[:], o_t[:], ew_ap(t2[:]))

                r_st.dma_start(out[i][:, c0:c0 + w], o_t[:])

            for i in range(n_tiles):
                # loads dedicated to the two HWDGE rings; the idle SWDGE
                # path absorbs two mid-ramp t2 loads (stores haven't
                # started yet) and the two HWDGE rings absorb the last
                # two stores during the drain (loads are done by then)
                r_t1 = nc.sync
                r_t2 = nc.scalar
                if i == n_tiles - 2:
                    r_st = nc.sync
                elif i == n_tiles - 1:
                    r_st = nc.scalar
                else:
                    r_st = nc.gpsimd
                emit_chunk(i, 0, tile_n, r_t1, r_t2, r_st)
    nc.finalize()
    return nc


def make_weights(w1, b1, w2, b2):
    """Host-side prep of the tiny conv weights (128 floats -> 2x[128,128])."""
    c = C
    wd = 0.5 * (w1[c:] - w2[c:])
    u = (w1[:c] + wd).astype(np.float32)
    v = (-w2[:c] + wd).astype(np.float32)
    u128 = np.zeros((P, P), np.float32)
    v128 = np.zeros((P, P), np.float32)
    for j in range(NCHUNK):
        blk = slice(j * c, (j + 1) * c)
        u128[blk, blk] = u[:, None]    # rows k = contraction, cols = bcast
        v128[blk, blk] = v[:, None]
    bias = np.full((P, 1), np.float32(b1[0]) - np.float32(b2[0]), np.float32)
    return u128, v128, bias


def to_tile_major(slab):
    """[C, S_CORE] -> [N_TILES, 128, TILE_N] with partition p = j*32 + c."""
    # (c, j, i, n) -> (i, jc, n)
    x = slab.reshape(C, NCHUNK, N_TILES, TILE_N)
    return np.ascontiguousarray(x.transpose(2, 1, 0, 3)).reshape(
        N_TILES, P, TILE_N)


def from_tile_major(tiles):
    """Inverse of to_tile_major."""
    x = tiles.reshape(N_TILES, NCHUNK, C, TILE_N)
    return np.ascontiguousarray(x.transpose(2, 1, 0, 3)).reshape(C, S_CORE)


def kernel(feature1, feature2, w1, b1, w2, b2):
    global LAST_RESULTS
    u128, v128, bias = make_weights(w1, b1, w2, b2)

    f1v = np.asarray(feature1, np.float32).reshape(B, C, S)
    f2v = np.asarray(feature2, np.float32).reshape(B, C, S)

    in_maps = []
    for k in range(N_CORES):
        b, q = divmod(k, QUARTERS)
        sl = slice(q * S_CORE, (q + 1) * S_CORE)
        in_maps.append({
            "f1": to_tile_major(f1v[b, :, sl]),
            "f2": to_tile_major(f2v[b, :, sl]),
            "cc": np.concatenate([u128, v128, bias], axis=1),
        })

    nc = build_nc()
    res = run_bass_kernel_spmd(nc, in_maps, list(range(N_CORES)))
    LAST_RESULTS = res

    shards = np.stack([from_tile_major(res.results[k]["out"])
                       for k in range(N_CORES)])
    full = (shards.reshape(B, QUARTERS, C, S_CORE)
                  .transpose(0, 2, 1, 3)
                  .reshape(B, C, D, H, W))
    return full.astype(np.float32)
